# revision 16
# baseline (speedup 1.0000x reference)
"""DeepBKT 4-layer transformer forward on 8 TRN2 NeuronCores.

Data-parallel over batch: B=16 -> 2 batch items per core; each core runs the
full 4-layer stack on its (2*512, 512) token slab.

v2 (fp16 + big-DMA + PE-dense scheduling):
  - all matmul operands fp16 (full-rate PE, ~2e-3 rel err vs the 2e-2 gate);
    psum accumulation and the softmax reciprocal stay fp32.
  - weights host-repacked per layer into contiguous (128, N) slabs so each
    weight tensor is ONE dma_start (5 DMAs/layer instead of ~215), prefetched
    one layer ahead (bufs=2 rings).
  - attention processed as 16 (batch, head) units, par-separate, with the
    scores matmul of block kb+1 issued before the AV matmul of block kb so
    the Act-engine exp latency is hidden; V projection of the NEXT layer is
    interleaved between units as PE filler (y stream is layer-invariant).
  - residual adds are identity matmuls accumulated into the same psum group
    as the O/FFN2 projections, so LayerNorm reads a single psum tile.
  - LN row stats via accum_out on the Act copy (sum) and a DVE
    square-scalar_tensor_tensor (sum of squares); normalization is a DVE
    tensor_scalar in bf16 (2x mode).
  - softmax denominators from a ones-column appended per head in v_aug
    (row 64 of the AV psum); reciprocal broadcast across partitions via a
    (1,64) ones matmul; strictly-causal masking multiplies exp(scores) of
    the 128x128 diagonal block only (Pool engine); key blocks beyond the
    query tile are never computed. Row 0 of each batch item ends all-zero
    (matches the reference's zero_pad).
"""

import numpy as np
import ml_dtypes

import concourse.bass as bass
import concourse.tile as tile
from concourse import bacc, mybir
from concourse.bass_utils import run_bass_kernel_spmd

F32 = mybir.dt.float32
BF = mybir.dt.float16
AF = mybir.ActivationFunctionType
OP = mybir.AluOpType

B, S, D, H, DFF, L = 16, 512, 512, 8, 2048, 4
DK = D // H                       # 64
NCORES = 8
BPC = B // NCORES                 # 2 batch items per core
T = BPC * S                       # 1024 tokens per core
NT = T // 128                     # 8 token tiles
NC = D // 128                     # 4 feature chunks
NF = DFF // 128                   # 16 ffn chunks
EPS = 1e-5
SCALE = 1.0 / np.sqrt(DK)

_CACHE = {}


def _build(flags):
    nc = bacc.Bacc("TRN2", target_bir_lowering=False, debug=False,
                   num_devices=NCORES)

    d = {}
    d["x0_d"] = nc.dram_tensor("x0", [T, D], BF, kind="ExternalInput")
    d["y0_d"] = nc.dram_tensor("y0", [T, D], BF, kind="ExternalInput")
    d["frs_d"] = nc.dram_tensor("frs", [128, T], BF, kind="ExternalInput")
    d["mask_d"] = nc.dram_tensor("mask01", [128, 128], BF, kind="ExternalInput")
    d["wk_d"] = nc.dram_tensor("Wk", [L, 128, NC * D], BF, kind="ExternalInput")
    d["wv_d"] = nc.dram_tensor("Wv", [L, 128, NC * D], BF, kind="ExternalInput")
    d["wo_d"] = nc.dram_tensor("Wo", [L, 128, NC * D], BF, kind="ExternalInput")
    d["w1_d"] = nc.dram_tensor("W1", [L, 128, NF * D], BF, kind="ExternalInput")
    d["w2_d"] = nc.dram_tensor("W2", [L, 128, NF * D], BF, kind="ExternalInput")
    d["bk_d"] = nc.dram_tensor("bk", [L, D, 1], F32, kind="ExternalInput")
    d["b1_d"] = nc.dram_tensor("b1", [L, DFF, 1], F32, kind="ExternalInput")
    # free-axis vectors pre-broadcast on host to (128, D)
    for nm in ("bvb", "bob", "b2b", "g1b", "be1b", "g2b", "be2b"):
        d[nm + "_d"] = nc.dram_tensor(nm, [L, 128, D], F32, kind="ExternalInput")
    d["out_d"] = nc.dram_tensor("out", [T, D], F32, kind="ExternalOutput")

    with tile.TileContext(nc) as tc:
        _emit(nc, tc, d, flags)
    nc.compile()
    return nc


def _emit(nc, tc, d, flags):
    use_bk, use_bv, use_bo, use_b1, use_b2, use_ln1, use_ln2 = flags
    import contextlib
    ctx = contextlib.ExitStack()
    with ctx:
        sb = ctx.enter_context(tc.tile_pool(name="sb", bufs=1))
        ps = ctx.enter_context(tc.tile_pool(name="ps", bufs=1, space="PSUM"))

        def tl(shape, dtype, tag, bufs, name=None):
            return sb.tile(shape, dtype, tag=tag, bufs=bufs, name=name or tag)

        def pA():
            return ps.tile([128, 512], F32, tag="psA", bufs=2, name="psA")

        def pB():
            return ps.tile([128, 512], F32, tag="psB", bufs=4, name="psB")

        def pT():
            return ps.tile([128, 512], BF, tag="psT", bufs=2, name="psT")

        # ---- constants ----
        frs_t = tl([128, T], BF, "frs", 1)
        nc.sync.dma_start(frs_t[:], d["frs_d"].ap())
        mask_t = tl([128, 128], BF, "mask", 1)
        nc.sync.dma_start(mask_t[:], d["mask_d"].ap())
        ones_f = tl([128, 64], BF, "onesf", 1)
        nc.gpsimd.memset(ones_f[:], 1.0)
        ones32 = tl([1, 64], F32, "ones32", 1)
        nc.gpsimd.memset(ones32[:], 1.0)
        eps_t = tl([128, 1], F32, "epsb", 1)
        nc.gpsimd.memset(eps_t[:], EPS)
        ident_t = tl([128, 128], BF, "ident", 1)
        nc.gpsimd.memset(ident_t[:], 0.0)
        nc.gpsimd.affine_select(
            out=ident_t[:], in_=ident_t[:], compare_op=OP.not_equal,
            fill=1.0, base=0, pattern=[[-1, 128]], channel_multiplier=1)

        # ---- weight loading (one DMA per tensor per layer, ring bufs=2) ----
        def load_weights(li):
            w = {}
            w["wk"] = tl([128, NC * D], BF, "wk", 2)
            w["wv"] = tl([128, NC * D], BF, "wv", 2)
            w["wo"] = tl([128, NC * D], BF, "wo", 2)
            w["w1"] = tl([128, NF * D], BF, "w1", 2)
            w["w2"] = tl([128, NF * D], BF, "w2", 2)
            for nm in ("wk", "wv", "wo", "w1", "w2"):
                nc.sync.dma_start(w[nm][:], d[nm + "_d"].ap()[li])
            if use_bk:
                w["bk"] = [tl([128, 1], F32, "bk", 8) for _ in range(NC)]
                for c in range(NC):
                    nc.sync.dma_start(w["bk"][c][:], d["bk_d"].ap()[li, c * 128:(c + 1) * 128, :])
            if use_bv:
                w["bv"] = tl([128, D], F32, "bvb", 2)
                nc.sync.dma_start(w["bv"][:], d["bvb_d"].ap()[li])
            if use_bo:
                w["bo"] = tl([128, D], F32, "bob", 2)
                nc.sync.dma_start(w["bo"][:], d["bob_d"].ap()[li])
            if use_b2:
                w["b2"] = tl([128, D], F32, "b2b", 2)
                nc.sync.dma_start(w["b2"][:], d["b2b_d"].ap()[li])
            if use_b1:
                w["b1"] = [tl([128, 1], F32, "b1f", 32) for _ in range(NF)]
                for f in range(NF):
                    nc.sync.dma_start(w["b1"][f][:], d["b1_d"].ap()[li, f * 128:(f + 1) * 128, :])
            if use_ln1:
                w["g1"] = tl([128, D], F32, "g1b", 2)
                w["be1"] = tl([128, D], F32, "be1b", 2)
                nc.sync.dma_start(w["g1"][:], d["g1b_d"].ap()[li])
                nc.sync.dma_start(w["be1"][:], d["be1b_d"].ap()[li])
            if use_ln2:
                w["g2"] = tl([128, D], F32, "g2b", 2)
                w["be2"] = tl([128, D], F32, "be2b", 2)
                nc.sync.dma_start(w["g2"][:], d["g2b_d"].ap()[li])
                nc.sync.dma_start(w["be2"][:], d["be2b_d"].ap()[li])
            return w

        # ---- transposes: 4 token tiles -> one psum bank -> chunk columns ----
        tp_flip = [0]

        def transpose_half(chunks, tiles, half, tag_hint=""):
            """tiles: 4 (128 tok, 512 feat) bf16 -> chunks[c][:, half*512:(half+1)*512]"""
            for c in range(NC):
                pt = pT()
                for i in range(4):
                    nc.tensor.transpose(
                        pt[:, i * 128:(i + 1) * 128],
                        tiles[i][:, c * 128:(c + 1) * 128], ident_t[:])
                dst = chunks[c][:, half * 512:(half + 1) * 512]
                if tp_flip[0] % 2 == 0:
                    nc.vector.tensor_copy(dst, pt[:])
                else:
                    nc.scalar.copy(dst, pt[:])
                tp_flip[0] += 1

        # ---- V projection for one token tile of a given layer ----
        def v_proj_tile(w, tt):
            va = tl([128, 8 * 65], BF, "vaug", 16, "vaug")
            pv = pA()
            for k in range(NC):
                nc.tensor.matmul(
                    pv[:, 0:512], yT[k][:, tt * 128:(tt + 1) * 128],
                    w["wv"][:, k * 512:(k + 1) * 512],
                    start=(k == 0), stop=(k == NC - 1))
            vdst = va[:].rearrange("p (g e) -> p g e", e=65)[:, :, 0:64]
            vsrc = pv[:, 0:512].rearrange("p (g e) -> p g e", e=64)
            if use_bv:
                nc.vector.scalar_tensor_tensor(
                    out=vdst, in0=vsrc, scalar=1.0,
                    in1=w["bv"][:].rearrange("p (g e) -> p g e", e=64),
                    op0=OP.mult, op1=OP.add)
            else:
                nc.scalar.copy(vdst, vsrc)
            nc.gpsimd.tensor_copy(
                va[:].rearrange("p (g e) -> p g e", e=65)[:, :, 64:65],
                ones_f[:, 0:8].rearrange("p (g e) -> p g e", e=1))
            return va

        # ---- initial loads + transposes ----
        x_tiles = [tl([128, D], BF, "x", 16, "x0t") for _ in range(NT)]
        for tt in range(NT):
            nc.sync.dma_start(x_tiles[tt][:], d["x0_d"].ap()[tt * 128:(tt + 1) * 128, :])
        w_cur = load_weights(0)

        yT = [tl([128, T], BF, "yT", 4, "yT") for _ in range(NC)]
        for half in range(2):
            y_tiles = [tl([128, D], BF, "y", 4, "y0t") for _ in range(4)]
            for i in range(4):
                tt = half * 4 + i
                nc.sync.dma_start(y_tiles[i][:], d["y0_d"].ap()[tt * 128:(tt + 1) * 128, :])
            transpose_half(yT, y_tiles, half)
        xT = [tl([128, T], BF, "xT", 8, "xT0") for _ in range(NC)]
        for half in range(2):
            transpose_half(xT, x_tiles[half * 4:(half + 1) * 4], half)

        # layer-0 V projections
        v_aug = [v_proj_tile(w_cur, tt) for tt in range(NT)]

        for li in range(L):
            w = w_cur
            w_next = None
            v_aug_next = [None] * NT
            if li + 1 < L:
                w_next = load_weights(li + 1)

            # ---- QK projection: qku (raw) / qks (fr*scale-scaled) ----
            qku = {}
            qks = {}
            for b in range(BPC):
                for c in range(NC):
                    pp = pA()
                    for k in range(NC):
                        nc.tensor.matmul(
                            pp[:, 0:512],
                            w["wk"][:, k * 512 + c * 128:k * 512 + (c + 1) * 128],
                            xT[k][:, b * 512:(b + 1) * 512],
                            start=(k == 0), stop=(k == NC - 1))
                    u = tl([128, 512], BF, "qku", 8)
                    s = tl([128, 512], BF, "qks", 8)
                    if use_bk:
                        nc.scalar.activation(u[:], pp[:, 0:512], AF.Identity,
                                             bias=w["bk"][c][:])
                        nc.vector.scalar_tensor_tensor(
                            out=s[:], in0=pp[:, 0:512], scalar=w["bk"][c][:],
                            in1=frs_t[:, b * 512:(b + 1) * 512],
                            op0=OP.add, op1=OP.mult)
                    else:
                        nc.vector.tensor_copy(u[:], pp[:, 0:512])
                        nc.vector.scalar_tensor_tensor(
                            out=s[:], in0=pp[:, 0:512], scalar=1.0,
                            in1=frs_t[:, b * 512:(b + 1) * 512],
                            op0=OP.mult, op1=OP.mult)
                    qku[(c, b)] = u
                    qks[(c, b)] = s

            # ---- attention: 16 par-separate units ----
            uoT = {}
            for b in range(BPC):
                for hp in range(NC):
                    uoT[(hp, b)] = tl([128, 512], BF, "uoT", 8, "uoT")
            pair_idx = 0
            for b in range(BPC):
                for hp in range(NC):
                    qu = qku[(hp, b)]
                    qs = qks[(hp, b)]
                    avs = [None, None]
                    for par in range(2):
                        r0 = par * 64
                        av = avs[par] = pB()
                        pg = [None] * 4
                        eT = [None] * 4

                        def S(kb):
                            n = 512 - 128 * kb
                            pg[kb] = pA()
                            nc.tensor.matmul(
                                pg[kb][:, 0:n],
                                qu[r0:r0 + 64, 128 * kb:128 * (kb + 1)],
                                qs[r0:r0 + 64, 128 * kb:512],
                                start=True, stop=True, tile_position=(r0, 0))
                            e = eT[kb] = tl([128, 512], BF, "eT", 5)
                            nc.scalar.activation(e[:, 0:n], pg[kb][:, 0:n], AF.Exp)
                            nc.gpsimd.tensor_tensor(
                                e[:, 0:128], e[:, 0:128], mask_t[:], OP.mult)

                        def A(kb):
                            n = 512 - 128 * kb
                            nc.tensor.matmul(
                                av[0:65, 128 * kb:512],
                                v_aug[b * 4 + kb][:, (2 * hp + par) * 65:(2 * hp + par + 1) * 65],
                                eT[kb][:, 0:n],
                                start=(kb == 0), stop=(kb == 3),
                                skip_group_check=True)

                        S(0); S(1); A(0); S(2); A(1); S(3); A(2); A(3)

                        # denominator row (bias keeps 1/denom inside fp16)
                        rrow = tl([1, 512], F32, "rrow", 2, "rrow")
                        nc.scalar.activation(rrow[:], av[64:65, 0:512],
                                             AF.Copy, bias=2e-5)
                        rinv = tl([1, 512], F32, "rinv", 2, "rinv")
                        nc.vector.reciprocal_approx_fast(rinv[:], rrow[:])
                        rinvh = tl([1, 512], BF, "rinvh", 2, "rinvh")
                        nc.gpsimd.tensor_copy(rinvh[:], rinv[:])
                        prb = pA()
                        nc.tensor.matmul(prb[0:64, 0:512], ones_f[0:1, :],
                                         rinvh[:],
                                         start=True, stop=True,
                                         tile_position=(0, 0))
                        uoU = tl([64, 512], BF, "uoU", 3, "uoU")
                        nc.scalar.copy(uoU[:], av[0:64, 0:512])
                        nc.vector.scalar_tensor_tensor(
                            out=uoT[(hp, b)][r0:r0 + 64, :],
                            in0=uoU[:], scalar=1.0,
                            in1=prb[0:64, 0:512],
                            op0=OP.mult, op1=OP.mult)

                    # PE filler: next layer's V projection, one tile per pair
                    if w_next is not None:
                        v_aug_next[pair_idx] = v_proj_tile(w_next, pair_idx)
                    pair_idx += 1

            # ---- O projection + residual + LN1 ----
            x_mid = [None] * NT
            for tt in range(NT):
                po = pA()
                for c in range(NC):
                    nc.tensor.matmul(
                        po[:, 0:512],
                        uoT[(c, tt // 4)][:, (tt % 4) * 128:(tt % 4 + 1) * 128],
                        w["wo"][:, c * 512:(c + 1) * 512],
                        start=(c == 0), stop=False)
                nc.tensor.matmul(
                    po[:, 0:512], ident_t[:], x_tiles[tt][:],
                    start=False, stop=True)
                if use_bo:
                    nc.vector.tensor_tensor(po[:, 0:512], po[:, 0:512],
                                            w["bo"][:], OP.add)
                x_mid[tt] = tl([128, D], BF, "x", 16, "xmid")
                _layernorm(nc, tl, po[:, 0:512], x_mid[tt],
                           (w["g1"], w["be1"]) if use_ln1 else None, eps_t)

            # ---- transpose x_mid -> xTm ----
            xTm = [tl([128, T], BF, "xT", 8, "xTm") for _ in range(NC)]
            for half in range(2):
                transpose_half(xTm, x_mid[half * 4:(half + 1) * 4], half)

            # ---- FFN + residual + LN2 ----
            x_out = [None] * NT
            last = li == L - 1
            for half in range(2):
                accs = [pB() for _ in range(4)]
                for f in range(NF):
                    ph = pA()
                    for k in range(NC):
                        nc.tensor.matmul(
                            ph[:, 0:512],
                            w["w1"][:, f * 512 + k * 128:f * 512 + (k + 1) * 128],
                            xTm[k][:, half * 512:(half + 1) * 512],
                            start=(k == 0), stop=(k == NC - 1))
                    hf = tl([128, 512], BF, "hf", 2)
                    if use_b1:
                        nc.scalar.activation(hf[:], ph[:, 0:512], AF.Relu,
                                             bias=w["b1"][f][:])
                    else:
                        nc.scalar.activation(hf[:], ph[:, 0:512], AF.Relu)
                    for q in range(4):
                        nc.tensor.matmul(
                            accs[q][:, 0:512], hf[:, q * 128:(q + 1) * 128],
                            w["w2"][:, f * 512:(f + 1) * 512],
                            start=(f == 0), stop=False)
                for q in range(4):
                    tt = half * 4 + q
                    nc.tensor.matmul(
                        accs[q][:, 0:512], ident_t[:], x_mid[tt][:],
                        start=False, stop=True)
                    if use_b2:
                        nc.vector.tensor_tensor(accs[q][:, 0:512],
                                                accs[q][:, 0:512],
                                                w["b2"][:], OP.add)
                    if last:
                        x_out[tt] = tl([128, D], F32, "xof", 2, "xout_f")
                    else:
                        x_out[tt] = tl([128, D], BF, "x", 16, "xout")
                    _layernorm(nc, tl, accs[q][:, 0:512], x_out[tt],
                               (w["g2"], w["be2"]) if use_ln2 else None, eps_t)
                    if last:
                        nc.sync.dma_start(
                            d["out_d"].ap()[tt * 128:(tt + 1) * 128, :],
                            x_out[tt][:])

            if not last:
                xT = [tl([128, T], BF, "xT", 8, "xTn") for _ in range(NC)]
                for half in range(2):
                    transpose_half(xT, x_out[half * 4:(half + 1) * 4], half)
                x_tiles = x_out
                v_aug = v_aug_next
                w_cur = w_next


def _layernorm(nc, tl, p_in, x_new, gb, eps_t):
    """x_new = LN(p_in) over the free dim (512); p_in already includes the
    residual (identity matmul accumulated into the psum group)."""
    xres = tl([128, D], BF, "xres", 3)
    msum = tl([128, 1], F32, "lnst", 14, "msum")
    nc.scalar.activation(xres[:], p_in, AF.Identity, accum_out=msum[:])
    scr = tl([128, D], BF, "lnscr", 1)
    sqsum = tl([128, 1], F32, "lnst", 14, "sqsum")
    nc.vector.scalar_tensor_tensor(
        out=scr[:], in0=xres[:], scalar=1.0, in1=xres[:],
        op0=OP.mult, op1=OP.mult, accum_out=sqsum[:])
    mu = tl([128, 1], F32, "lnst", 14, "mu")
    nc.vector.tensor_scalar_mul(mu[:], msum[:], 1.0 / D)
    var = tl([128, 1], F32, "lnst", 14, "var")
    nc.vector.tensor_scalar(
        out=var[:], in0=sqsum[:], scalar1=1.0 / D, scalar2=None, op0=OP.mult)
    nc.vector.scalar_tensor_tensor(
        out=var[:], in0=mu[:], scalar=mu[:], in1=var[:], op0=OP.mult,
        op1=OP.subtract)
    nc.vector.tensor_scalar_mul(var[:], var[:], -1.0)
    sd = tl([128, 1], F32, "lnst", 14, "sd")
    nc.scalar.activation(sd[:], var[:], AF.Sqrt, bias=eps_t[:])
    rstd = tl([128, 1], F32, "lnst", 14, "rstd")
    nc.vector.reciprocal(rstd[:], sd[:])
    negmu = tl([128, 1], F32, "lnst", 14, "negmu")
    nc.vector.tensor_scalar_mul(negmu[:], mu[:], -1.0)
    if gb is None:
        nc.vector.tensor_scalar(
            out=x_new[:], in0=xres[:], scalar1=negmu[:], scalar2=rstd[:],
            op0=OP.add, op1=OP.mult)
    else:
        g_t, be_t = gb
        xn = tl([128, D], F32, "xn", 2)
        nc.vector.tensor_scalar(
            out=xn[:], in0=xres[:], scalar1=negmu[:], scalar2=rstd[:],
            op0=OP.add, op1=OP.mult)
        nc.vector.tensor_tensor(xn[:], xn[:], g_t[:], OP.mult)
        nc.vector.tensor_tensor(x_new[:], xn[:], be_t[:], OP.add)


def _host_prep(inputs):
    bf = np.float16
    q = np.asarray(inputs["q_embed"], np.float32)
    qa = np.asarray(inputs["qa_embed"], np.float32)
    fr = np.asarray(inputs["forget_rate"], np.float32)
    pe = np.asarray(inputs["pe"], np.float32)
    x0 = (q + pe).astype(bf)
    y0 = (qa + pe).astype(bf)

    flags = (
        bool(np.any(inputs["bk"])), bool(np.any(inputs["bv"])),
        bool(np.any(inputs["bo"])), bool(np.any(inputs["b1"])),
        bool(np.any(inputs["b2"])),
        bool(np.any(np.asarray(inputs["ln1_g"]) != 1.0) or np.any(inputs["ln1_b"])),
        bool(np.any(np.asarray(inputs["ln2_g"]) != 1.0) or np.any(inputs["ln2_b"])),
    )

    mask01 = (np.arange(128)[None, :] > np.arange(128)[:, None]).astype(bf)

    def bcast(v):  # (L, D) -> (L, 128, D)
        v = np.asarray(v, np.float32)
        return np.ascontiguousarray(np.broadcast_to(v[:, None, :], (L, 128, v.shape[-1])))

    Wk = np.asarray(inputs["Wk"], np.float32)
    Wv = np.asarray(inputs["Wv"], np.float32)
    Wo = np.asarray(inputs["Wo"], np.float32)
    W1 = np.asarray(inputs["W1"], np.float32)
    W2 = np.asarray(inputs["W2"], np.float32)

    def pack_dd(Wm):  # (L, 512, 512) -> (L, 128, 4*512): [p, k*512+j]
        return np.ascontiguousarray(
            Wm.reshape(L, NC, 128, D).transpose(0, 2, 1, 3).reshape(L, 128, NC * D)
        ).astype(bf)

    # W1 (L, 512, 2048) -> (L, 128, 16*512): [p, f*512 + k*128 + j]
    w1p = np.ascontiguousarray(
        W1.reshape(L, NC, 128, NF, 128).transpose(0, 2, 3, 1, 4).reshape(L, 128, NF * D)
    ).astype(bf)
    # W2 (L, 2048, 512) -> (L, 128, 16*512): [p, f*512 + j]
    w2p = np.ascontiguousarray(
        W2.reshape(L, NF, 128, D).transpose(0, 2, 1, 3).reshape(L, 128, NF * D)
    ).astype(bf)

    common = {
        "Wk": pack_dd(Wk), "Wv": pack_dd(Wv), "Wo": pack_dd(Wo),
        "W1": w1p, "W2": w2p,
        "bk": np.ascontiguousarray(inputs["bk"], np.float32).reshape(L, D, 1),
        "b1": np.ascontiguousarray(inputs["b1"], np.float32).reshape(L, DFF, 1),
        "bvb": bcast(inputs["bv"]), "bob": bcast(inputs["bo"]),
        "b2b": bcast(inputs["b2"]),
        "g1b": bcast(inputs["ln1_g"]), "be1b": bcast(inputs["ln1_b"]),
        "g2b": bcast(inputs["ln2_g"]), "be2b": bcast(inputs["ln2_b"]),
        "mask01": mask01,
    }

    in_maps = []
    for c in range(NCORES):
        sl = slice(c * BPC, (c + 1) * BPC)
        frs = (fr[sl, :, 0].reshape(1, T) * SCALE).astype(bf)
        m = dict(common)
        m["x0"] = np.ascontiguousarray(x0[sl].reshape(T, D))
        m["y0"] = np.ascontiguousarray(y0[sl].reshape(T, D))
        m["frs"] = np.ascontiguousarray(np.broadcast_to(frs, (128, T)))
        in_maps.append(m)
    return in_maps, flags


def kernel(_trace=False, **inputs):
    in_maps, flags = _host_prep(inputs)
    if flags not in _CACHE:
        _CACHE[flags] = _build(flags)
    nc = _CACHE[flags]
    br = run_bass_kernel_spmd(nc, in_maps, list(range(NCORES)), trace=_trace)
    out = np.empty((B, S, D), np.float32)
    for c in range(NCORES):
        out[c * BPC:(c + 1) * BPC] = br.results[c]["out"].reshape(BPC, S, D)
    if _trace:
        kernel.last_result = br
    return out


# revision 17
# speedup vs baseline: 1.3032x; 1.3032x over previous
"""DeepBKT 4-layer transformer forward on 8 TRN2 NeuronCores.

Data-parallel over batch: B=16 -> 2 batch items per core; each core runs the
full 4-layer stack on its (2*512, 512) token slab.

v2 (fp16 + big-DMA + PE-dense scheduling):
  - all matmul operands fp16 (full-rate PE, ~2e-3 rel err vs the 2e-2 gate);
    psum accumulation and the softmax reciprocal stay fp32.
  - weights host-repacked per layer into contiguous (128, N) slabs so each
    weight tensor is ONE dma_start (5 DMAs/layer instead of ~215), prefetched
    one layer ahead (bufs=2 rings).
  - attention processed as 16 (batch, head) units, par-separate, with the
    scores matmul of block kb+1 issued before the AV matmul of block kb so
    the Act-engine exp latency is hidden; V projection of the NEXT layer is
    interleaved between units as PE filler (y stream is layer-invariant).
  - residual adds are identity matmuls accumulated into the same psum group
    as the O/FFN2 projections, so LayerNorm reads a single psum tile.
  - LN row stats via accum_out on the Act copy (sum) and a DVE
    square-scalar_tensor_tensor (sum of squares); normalization is a DVE
    tensor_scalar in bf16 (2x mode).
  - softmax denominators from a ones-column appended per head in v_aug
    (row 64 of the AV psum); reciprocal broadcast across partitions via a
    (1,64) ones matmul; strictly-causal masking multiplies exp(scores) of
    the 128x128 diagonal block only (Pool engine); key blocks beyond the
    query tile are never computed. Row 0 of each batch item ends all-zero
    (matches the reference's zero_pad).
"""

import numpy as np
import ml_dtypes

import concourse.bass as bass
import concourse.tile as tile
from concourse import bacc, mybir
from concourse.bass_utils import run_bass_kernel_spmd

F32 = mybir.dt.float32
BF = mybir.dt.float16
AF = mybir.ActivationFunctionType
OP = mybir.AluOpType

B, S, D, H, DFF, L = 16, 512, 512, 8, 2048, 4
DK = D // H                       # 64
NCORES = 8
BPC = B // NCORES                 # 2 batch items per core
T = BPC * S                       # 1024 tokens per core
NT = T // 128                     # 8 token tiles
NC = D // 128                     # 4 feature chunks
NF = DFF // 128                   # 16 ffn chunks
EPS = 1e-5
SCALE = 1.0 / np.sqrt(DK)

_CACHE = {}


def _build(flags):
    nc = bacc.Bacc("TRN2", target_bir_lowering=False, debug=False,
                   num_devices=NCORES)

    d = {}
    d["x0_d"] = nc.dram_tensor("x0", [T, D], BF, kind="ExternalInput")
    d["y0_d"] = nc.dram_tensor("y0", [T, D], BF, kind="ExternalInput")
    d["frs_d"] = nc.dram_tensor("frs", [128, T], BF, kind="ExternalInput")
    d["mask_d"] = nc.dram_tensor("mask01", [128, 128], BF, kind="ExternalInput")
    d["wk_d"] = nc.dram_tensor("Wk", [L, 128, NC * D], BF, kind="ExternalInput")
    d["wv_d"] = nc.dram_tensor("Wv", [L, 128, NC * D], BF, kind="ExternalInput")
    d["wo_d"] = nc.dram_tensor("Wo", [L, 128, NC * D], BF, kind="ExternalInput")
    d["w1_d"] = nc.dram_tensor("W1", [L, 128, NF * D], BF, kind="ExternalInput")
    d["w2_d"] = nc.dram_tensor("W2", [L, 128, NF * D], BF, kind="ExternalInput")
    d["bk_d"] = nc.dram_tensor("bk", [L, D, 1], F32, kind="ExternalInput")
    d["b1_d"] = nc.dram_tensor("b1", [L, DFF, 1], F32, kind="ExternalInput")
    # free-axis vectors pre-broadcast on host to (128, D)
    for nm in ("bvb", "bob", "b2b", "g1b", "be1b", "g2b", "be2b"):
        d[nm + "_d"] = nc.dram_tensor(nm, [L, 128, D], F32, kind="ExternalInput")
    d["out_d"] = nc.dram_tensor("out", [T, D], F32, kind="ExternalOutput")

    with tile.TileContext(nc) as tc:
        _emit(nc, tc, d, flags)
    nc.compile()
    return nc


def _emit(nc, tc, d, flags):
    use_bk, use_bv, use_bo, use_b1, use_b2, use_ln1, use_ln2 = flags
    import contextlib
    ctx = contextlib.ExitStack()
    with ctx:
        sb = ctx.enter_context(tc.tile_pool(name="sb", bufs=1))
        ps = ctx.enter_context(tc.tile_pool(name="ps", bufs=1, space="PSUM"))

        def tl(shape, dtype, tag, bufs, name=None):
            return sb.tile(shape, dtype, tag=tag, bufs=bufs, name=name or tag)

        def pA():
            return ps.tile([128, 512], F32, tag="psA", bufs=2, name="psA")

        def pB():
            return ps.tile([128, 512], F32, tag="psB", bufs=4, name="psB")

        def pT():
            return ps.tile([128, 512], BF, tag="psT", bufs=2, name="psT")

        # ---- constants ----
        frs_t = tl([128, T], BF, "frs", 1)
        nc.sync.dma_start(frs_t[:], d["frs_d"].ap())
        mask_t = tl([128, 128], BF, "mask", 1)
        nc.sync.dma_start(mask_t[:], d["mask_d"].ap())
        ones_f = tl([128, 64], BF, "onesf", 1)
        nc.gpsimd.memset(ones_f[:], 1.0)
        ones32 = tl([1, 64], F32, "ones32", 1)
        nc.gpsimd.memset(ones32[:], 1.0)
        eps_t = tl([128, 1], F32, "epsb", 1)
        nc.gpsimd.memset(eps_t[:], EPS)
        ident_t = tl([128, 128], BF, "ident", 1)
        nc.gpsimd.memset(ident_t[:], 0.0)
        nc.gpsimd.affine_select(
            out=ident_t[:], in_=ident_t[:], compare_op=OP.not_equal,
            fill=1.0, base=0, pattern=[[-1, 128]], channel_multiplier=1)

        # ---- weight loading (one DMA per tensor per layer, ring bufs=2) ----
        def load_weights(li):
            w = {}
            w["wk"] = tl([128, NC * D], BF, "wk", 2)
            w["wv"] = tl([128, NC * D], BF, "wv", 2)
            w["wo"] = tl([128, NC * D], BF, "wo", 2)
            w["w1"] = tl([128, NF * D], BF, "w1", 2)
            w["w2"] = tl([128, NF * D], BF, "w2", 2)
            for nm in ("wk", "wv", "wo", "w1", "w2"):
                nc.sync.dma_start(w[nm][:], d[nm + "_d"].ap()[li])
            if use_bk:
                w["bk"] = [tl([128, 1], F32, "bk", 8) for _ in range(NC)]
                for c in range(NC):
                    nc.sync.dma_start(w["bk"][c][:], d["bk_d"].ap()[li, c * 128:(c + 1) * 128, :])
            if use_bv:
                w["bv"] = tl([128, D], F32, "bvb", 2)
                nc.sync.dma_start(w["bv"][:], d["bvb_d"].ap()[li])
            if use_bo:
                w["bo"] = tl([128, D], F32, "bob", 2)
                nc.sync.dma_start(w["bo"][:], d["bob_d"].ap()[li])
            if use_b2:
                w["b2"] = tl([128, D], F32, "b2b", 2)
                nc.sync.dma_start(w["b2"][:], d["b2b_d"].ap()[li])
            if use_b1:
                w["b1"] = [tl([128, 1], F32, "b1f", 32) for _ in range(NF)]
                for f in range(NF):
                    nc.sync.dma_start(w["b1"][f][:], d["b1_d"].ap()[li, f * 128:(f + 1) * 128, :])
            if use_ln1:
                w["g1"] = tl([128, D], F32, "g1b", 2)
                w["be1"] = tl([128, D], F32, "be1b", 2)
                nc.sync.dma_start(w["g1"][:], d["g1b_d"].ap()[li])
                nc.sync.dma_start(w["be1"][:], d["be1b_d"].ap()[li])
            if use_ln2:
                w["g2"] = tl([128, D], F32, "g2b", 2)
                w["be2"] = tl([128, D], F32, "be2b", 2)
                nc.sync.dma_start(w["g2"][:], d["g2b_d"].ap()[li])
                nc.sync.dma_start(w["be2"][:], d["be2b_d"].ap()[li])
            return w

        # ---- transposes: 4 token tiles -> one psum bank -> chunk columns ----
        tp_flip = [0]

        def transpose_half(chunks, tiles, half, tag_hint=""):
            """tiles: 4 (128 tok, 512 feat) bf16 -> chunks[c][:, half*512:(half+1)*512]"""
            for c in range(NC):
                pt = pT()
                for i in range(4):
                    nc.tensor.transpose(
                        pt[:, i * 128:(i + 1) * 128],
                        tiles[i][:, c * 128:(c + 1) * 128], ident_t[:])
                dst = chunks[c][:, half * 512:(half + 1) * 512]
                if tp_flip[0] % 2 == 0:
                    nc.vector.tensor_copy(dst, pt[:])
                else:
                    nc.scalar.copy(dst, pt[:])
                tp_flip[0] += 1

        # ---- V projection for one token tile of a given layer ----
        def v_proj_tile(w, tt):
            va = tl([128, 8 * 65], BF, "vaug", 16, "vaug")
            pv = pB()
            for k in range(NC):
                nc.tensor.matmul(
                    pv[:, 0:512], yT[k][:, tt * 128:(tt + 1) * 128],
                    w["wv"][:, k * 512:(k + 1) * 512],
                    start=(k == 0), stop=(k == NC - 1))
            vdst = va[:].rearrange("p (g e) -> p g e", e=65)[:, :, 0:64]
            vsrc = pv[:, 0:512].rearrange("p (g e) -> p g e", e=64)
            if use_bv:
                nc.vector.scalar_tensor_tensor(
                    out=vdst, in0=vsrc, scalar=1.0,
                    in1=w["bv"][:].rearrange("p (g e) -> p g e", e=64),
                    op0=OP.mult, op1=OP.add)
            else:
                nc.scalar.copy(vdst, vsrc)
            nc.gpsimd.tensor_copy(
                va[:].rearrange("p (g e) -> p g e", e=65)[:, :, 64:65],
                ones_f[:, 0:8].rearrange("p (g e) -> p g e", e=1))
            return va

        # ---- initial loads + transposes ----
        x_tiles = [tl([128, D], BF, "x", 16, "x0t") for _ in range(NT)]
        for tt in range(NT):
            nc.sync.dma_start(x_tiles[tt][:], d["x0_d"].ap()[tt * 128:(tt + 1) * 128, :])
        w_cur = load_weights(0)

        yT = [tl([128, T], BF, "yT", 4, "yT") for _ in range(NC)]
        for half in range(2):
            y_tiles = [tl([128, D], BF, "y", 4, "y0t") for _ in range(4)]
            for i in range(4):
                tt = half * 4 + i
                nc.sync.dma_start(y_tiles[i][:], d["y0_d"].ap()[tt * 128:(tt + 1) * 128, :])
            transpose_half(yT, y_tiles, half)
        xT = [tl([128, T], BF, "xT", 8, "xT0") for _ in range(NC)]
        for half in range(2):
            transpose_half(xT, x_tiles[half * 4:(half + 1) * 4], half)

        # layer-0 V projections
        v_aug = [v_proj_tile(w_cur, tt) for tt in range(NT)]

        for li in range(L):
            w = w_cur
            w_next = None
            v_aug_next = [None] * NT
            if li + 1 < L:
                w_next = load_weights(li + 1)

            # ---- QK projection: qku (raw) / qks (fr*scale-scaled) ----
            qku = {}
            qks = {}
            for b in range(BPC):
                for c in range(NC):
                    pp = pA()
                    for k in range(NC):
                        nc.tensor.matmul(
                            pp[:, 0:512],
                            w["wk"][:, k * 512 + c * 128:k * 512 + (c + 1) * 128],
                            xT[k][:, b * 512:(b + 1) * 512],
                            start=(k == 0), stop=(k == NC - 1))
                    u = tl([128, 512], BF, "qku", 8)
                    s = tl([128, 512], BF, "qks", 8)
                    if use_bk:
                        nc.scalar.activation(u[:], pp[:, 0:512], AF.Identity,
                                             bias=w["bk"][c][:])
                        nc.vector.scalar_tensor_tensor(
                            out=s[:], in0=pp[:, 0:512], scalar=w["bk"][c][:],
                            in1=frs_t[:, b * 512:(b + 1) * 512],
                            op0=OP.add, op1=OP.mult)
                    else:
                        nc.vector.tensor_copy(u[:], pp[:, 0:512])
                        nc.vector.scalar_tensor_tensor(
                            out=s[:], in0=pp[:, 0:512], scalar=1.0,
                            in1=frs_t[:, b * 512:(b + 1) * 512],
                            op0=OP.mult, op1=OP.mult)
                    qku[(c, b)] = u
                    qks[(c, b)] = s

            # ---- attention: 16 par-separate units ----
            uoT = {}
            for b in range(BPC):
                for hp in range(NC):
                    uoT[(hp, b)] = tl([128, 512], BF, "uoT", 8, "uoT")
            pair_idx = 0
            for b in range(BPC):
                for hp in range(NC):
                    qu = qku[(hp, b)]
                    qs = qks[(hp, b)]
                    avs = [None, None]
                    for par in range(2):
                        r0 = par * 64
                        av = avs[par] = pB()
                        pg = [None] * 4
                        eT = [None] * 4

                        def S(kb):
                            n = 512 - 128 * kb
                            pg[kb] = pA()
                            nc.tensor.matmul(
                                pg[kb][:, 0:n],
                                qu[r0:r0 + 64, 128 * kb:128 * (kb + 1)],
                                qs[r0:r0 + 64, 128 * kb:512],
                                start=True, stop=True, tile_position=(r0, 0))
                            e = eT[kb] = tl([128, 512], BF, "eT", 5)
                            nc.scalar.activation(e[:, 0:n], pg[kb][:, 0:n], AF.Exp)
                            nc.vector.tensor_tensor(
                                e[:, 0:128], e[:, 0:128], mask_t[:], OP.mult)

                        def A(kb):
                            n = 512 - 128 * kb
                            nc.tensor.matmul(
                                av[0:65, 128 * kb:512],
                                v_aug[b * 4 + kb][:, (2 * hp + par) * 65:(2 * hp + par + 1) * 65],
                                eT[kb][:, 0:n],
                                start=(kb == 0), stop=(kb == 3),
                                skip_group_check=True)

                        S(0); S(1); A(0); S(2); A(1); S(3); A(2); A(3)

                        # denominator row (bias keeps 1/denom inside fp16)
                        rrow = tl([1, 512], F32, "rrow", 2, "rrow")
                        nc.scalar.activation(rrow[:], av[64:65, 0:512],
                                             AF.Copy, bias=2e-5)
                        rinv = tl([1, 512], F32, "rinv", 2, "rinv")
                        nc.vector.reciprocal_approx_fast(rinv[:], rrow[:])
                        rinvh = tl([1, 512], BF, "rinvh", 2, "rinvh")
                        nc.vector.tensor_copy(rinvh[:], rinv[:])
                        prb = pB()
                        nc.tensor.matmul(prb[0:64, 0:512], ones_f[0:1, :],
                                         rinvh[:],
                                         start=True, stop=True,
                                         tile_position=(0, 0))
                        uoU = tl([64, 512], BF, "uoU", 3, "uoU")
                        nc.scalar.copy(uoU[:], av[0:64, 0:512])
                        nc.vector.scalar_tensor_tensor(
                            out=uoT[(hp, b)][r0:r0 + 64, :],
                            in0=uoU[:], scalar=1.0,
                            in1=prb[0:64, 0:512],
                            op0=OP.mult, op1=OP.mult)

                    # PE filler: next layer's V projection, one tile per pair
                    if w_next is not None:
                        v_aug_next[pair_idx] = v_proj_tile(w_next, pair_idx)
                    pair_idx += 1

            # ---- O projection + residual + LN1 ----
            x_mid = [None] * NT
            for tt in range(NT):
                po = pA()
                for c in range(NC):
                    nc.tensor.matmul(
                        po[:, 0:512],
                        uoT[(c, tt // 4)][:, (tt % 4) * 128:(tt % 4 + 1) * 128],
                        w["wo"][:, c * 512:(c + 1) * 512],
                        start=(c == 0), stop=False)
                nc.tensor.matmul(
                    po[:, 0:512], ident_t[:], x_tiles[tt][:],
                    start=False, stop=True)
                if use_bo:
                    nc.vector.tensor_tensor(po[:, 0:512], po[:, 0:512],
                                            w["bo"][:], OP.add)
                x_mid[tt] = tl([128, D], BF, "x", 16, "xmid")
                _layernorm(nc, tl, po[:, 0:512], x_mid[tt],
                           (w["g1"], w["be1"]) if use_ln1 else None, eps_t)

            # ---- transpose x_mid -> xTm ----
            xTm = [tl([128, T], BF, "xT", 8, "xTm") for _ in range(NC)]
            for half in range(2):
                transpose_half(xTm, x_mid[half * 4:(half + 1) * 4], half)

            # ---- FFN + residual + LN2 ----
            x_out = [None] * NT
            last = li == L - 1
            for half in range(2):
                accs = [pB() for _ in range(4)]
                for f in range(NF):
                    ph = pA()
                    for k in range(NC):
                        nc.tensor.matmul(
                            ph[:, 0:512],
                            w["w1"][:, f * 512 + k * 128:f * 512 + (k + 1) * 128],
                            xTm[k][:, half * 512:(half + 1) * 512],
                            start=(k == 0), stop=(k == NC - 1))
                    hf = tl([128, 512], BF, "hf", 2)
                    if use_b1:
                        nc.scalar.activation(hf[:], ph[:, 0:512], AF.Relu,
                                             bias=w["b1"][f][:])
                    else:
                        nc.scalar.activation(hf[:], ph[:, 0:512], AF.Relu)
                    for q in range(4):
                        nc.tensor.matmul(
                            accs[q][:, 0:512], hf[:, q * 128:(q + 1) * 128],
                            w["w2"][:, f * 512:(f + 1) * 512],
                            start=(f == 0), stop=False)
                for q in range(4):
                    tt = half * 4 + q
                    nc.tensor.matmul(
                        accs[q][:, 0:512], ident_t[:], x_mid[tt][:],
                        start=False, stop=True)
                    if use_b2:
                        nc.vector.tensor_tensor(accs[q][:, 0:512],
                                                accs[q][:, 0:512],
                                                w["b2"][:], OP.add)
                    if last:
                        x_out[tt] = tl([128, D], F32, "xof", 2, "xout_f")
                    else:
                        x_out[tt] = tl([128, D], BF, "x", 16, "xout")
                    _layernorm(nc, tl, accs[q][:, 0:512], x_out[tt],
                               (w["g2"], w["be2"]) if use_ln2 else None, eps_t)
                    if last:
                        nc.sync.dma_start(
                            d["out_d"].ap()[tt * 128:(tt + 1) * 128, :],
                            x_out[tt][:])

            if not last:
                xT = [tl([128, T], BF, "xT", 8, "xTn") for _ in range(NC)]
                for half in range(2):
                    transpose_half(xT, x_out[half * 4:(half + 1) * 4], half)
                x_tiles = x_out
                v_aug = v_aug_next
                w_cur = w_next


def _layernorm(nc, tl, p_in, x_new, gb, eps_t):
    """x_new = LN(p_in) over the free dim (512); p_in already includes the
    residual (identity matmul accumulated into the psum group)."""
    xres = tl([128, D], BF, "xres", 3)
    msum = tl([128, 1], F32, "lnst", 14, "msum")
    nc.scalar.activation(xres[:], p_in, AF.Identity, accum_out=msum[:])
    scr = tl([128, D], BF, "lnscr", 1)
    sqsum = tl([128, 1], F32, "lnst", 14, "sqsum")
    nc.vector.scalar_tensor_tensor(
        out=scr[:], in0=xres[:], scalar=1.0, in1=xres[:],
        op0=OP.mult, op1=OP.mult, accum_out=sqsum[:])
    mu = tl([128, 1], F32, "lnst", 14, "mu")
    nc.vector.tensor_scalar_mul(mu[:], msum[:], 1.0 / D)
    var = tl([128, 1], F32, "lnst", 14, "var")
    nc.vector.tensor_scalar(
        out=var[:], in0=sqsum[:], scalar1=1.0 / D, scalar2=None, op0=OP.mult)
    nc.vector.scalar_tensor_tensor(
        out=var[:], in0=mu[:], scalar=mu[:], in1=var[:], op0=OP.mult,
        op1=OP.subtract)
    nc.vector.tensor_scalar_mul(var[:], var[:], -1.0)
    sd = tl([128, 1], F32, "lnst", 14, "sd")
    nc.scalar.activation(sd[:], var[:], AF.Sqrt, bias=eps_t[:])
    rstd = tl([128, 1], F32, "lnst", 14, "rstd")
    nc.vector.reciprocal(rstd[:], sd[:])
    negmu = tl([128, 1], F32, "lnst", 14, "negmu")
    nc.vector.tensor_scalar_mul(negmu[:], mu[:], -1.0)
    if gb is None:
        nc.vector.tensor_scalar(
            out=x_new[:], in0=xres[:], scalar1=negmu[:], scalar2=rstd[:],
            op0=OP.add, op1=OP.mult)
    else:
        g_t, be_t = gb
        xn = tl([128, D], F32, "xn", 2)
        nc.vector.tensor_scalar(
            out=xn[:], in0=xres[:], scalar1=negmu[:], scalar2=rstd[:],
            op0=OP.add, op1=OP.mult)
        nc.vector.tensor_tensor(xn[:], xn[:], g_t[:], OP.mult)
        nc.vector.tensor_tensor(x_new[:], xn[:], be_t[:], OP.add)


def _host_prep(inputs):
    bf = np.float16
    q = np.asarray(inputs["q_embed"], np.float32)
    qa = np.asarray(inputs["qa_embed"], np.float32)
    fr = np.asarray(inputs["forget_rate"], np.float32)
    pe = np.asarray(inputs["pe"], np.float32)
    x0 = (q + pe).astype(bf)
    y0 = (qa + pe).astype(bf)

    flags = (
        bool(np.any(inputs["bk"])), bool(np.any(inputs["bv"])),
        bool(np.any(inputs["bo"])), bool(np.any(inputs["b1"])),
        bool(np.any(inputs["b2"])),
        bool(np.any(np.asarray(inputs["ln1_g"]) != 1.0) or np.any(inputs["ln1_b"])),
        bool(np.any(np.asarray(inputs["ln2_g"]) != 1.0) or np.any(inputs["ln2_b"])),
    )

    mask01 = (np.arange(128)[None, :] > np.arange(128)[:, None]).astype(bf)

    def bcast(v):  # (L, D) -> (L, 128, D)
        v = np.asarray(v, np.float32)
        return np.ascontiguousarray(np.broadcast_to(v[:, None, :], (L, 128, v.shape[-1])))

    Wk = np.asarray(inputs["Wk"], np.float32)
    Wv = np.asarray(inputs["Wv"], np.float32)
    Wo = np.asarray(inputs["Wo"], np.float32)
    W1 = np.asarray(inputs["W1"], np.float32)
    W2 = np.asarray(inputs["W2"], np.float32)

    def pack_dd(Wm):  # (L, 512, 512) -> (L, 128, 4*512): [p, k*512+j]
        return np.ascontiguousarray(
            Wm.reshape(L, NC, 128, D).transpose(0, 2, 1, 3).reshape(L, 128, NC * D)
        ).astype(bf)

    # W1 (L, 512, 2048) -> (L, 128, 16*512): [p, f*512 + k*128 + j]
    w1p = np.ascontiguousarray(
        W1.reshape(L, NC, 128, NF, 128).transpose(0, 2, 3, 1, 4).reshape(L, 128, NF * D)
    ).astype(bf)
    # W2 (L, 2048, 512) -> (L, 128, 16*512): [p, f*512 + j]
    w2p = np.ascontiguousarray(
        W2.reshape(L, NF, 128, D).transpose(0, 2, 1, 3).reshape(L, 128, NF * D)
    ).astype(bf)

    common = {
        "Wk": pack_dd(Wk), "Wv": pack_dd(Wv), "Wo": pack_dd(Wo),
        "W1": w1p, "W2": w2p,
        "bk": np.ascontiguousarray(inputs["bk"], np.float32).reshape(L, D, 1),
        "b1": np.ascontiguousarray(inputs["b1"], np.float32).reshape(L, DFF, 1),
        "bvb": bcast(inputs["bv"]), "bob": bcast(inputs["bo"]),
        "b2b": bcast(inputs["b2"]),
        "g1b": bcast(inputs["ln1_g"]), "be1b": bcast(inputs["ln1_b"]),
        "g2b": bcast(inputs["ln2_g"]), "be2b": bcast(inputs["ln2_b"]),
        "mask01": mask01,
    }

    in_maps = []
    for c in range(NCORES):
        sl = slice(c * BPC, (c + 1) * BPC)
        frs = (fr[sl, :, 0].reshape(1, T) * SCALE).astype(bf)
        m = dict(common)
        m["x0"] = np.ascontiguousarray(x0[sl].reshape(T, D))
        m["y0"] = np.ascontiguousarray(y0[sl].reshape(T, D))
        m["frs"] = np.ascontiguousarray(np.broadcast_to(frs, (128, T)))
        in_maps.append(m)
    return in_maps, flags


def kernel(_trace=False, **inputs):
    in_maps, flags = _host_prep(inputs)
    if flags not in _CACHE:
        _CACHE[flags] = _build(flags)
    nc = _CACHE[flags]
    br = run_bass_kernel_spmd(nc, in_maps, list(range(NCORES)), trace=_trace)
    out = np.empty((B, S, D), np.float32)
    for c in range(NCORES):
        out[c * BPC:(c + 1) * BPC] = br.results[c]["out"].reshape(BPC, S, D)
    if _trace:
        kernel.last_result = br
    return out
